# revision 43
# baseline (speedup 1.0000x reference)
"""MoE (dense all-expert FFN with double-softmax routing) on 8 trn2 NeuronCores.

Expert-parallel: core c holds expert c's W1/W2/b1/b2 resident in SBUF (fp8e4,
pre-scaled x64 on host for precision) and computes its expert's routing-
weighted contribution
    contrib_c = weight_c * mask_c * (swish(x @ W1[c] + b1[c]) @ W2[c] + b2[c])
for all 4096 tokens, written transposed as [128, 8, 4096] bf16.  The host
gathers the 8 partial outputs and forms  sum_c(contrib_c)^T + x  (a pure
8-way reduction + residual + layout transform; all matmuls / softmaxes /
activations / masking run on device).

Both big matmuls run in fp8 DoubleRow mode (2 fp8 weights per PE cell,
contracting 256 rows per instruction, ~2x bf16 throughput).  The x64 weight
scale is undone by the activation `scale` for mm1 and folded into the
routing-weight broadcast for mm2.  The router also runs on the fp8 x / Wr
(validated: identical >0.1 mask as the f32 reference, with ~8x margin) with a
bf16 softmax chain; the weighted combine multiplier stays f32.

Structure per token tile bt (512 tokens): the kernel is software-pipelined as
  router(bt+1), mm1(bt+1) -> h8(bt+1)   then   mm2(bt) -> combine -> out DMA
so the ScalarE activation backlog of mm1(bt+1) drains during mm2(bt)'s PE
work instead of stalling it.  The router's logit matmul also runs DoubleRow
(Wr zero-padded 8 -> 16 experts so the weight AP middle step is 16 bytes).

All tensors are staged host-side so every DMA is per-partition contiguous
with 2-4KB descriptors (bt-major x/out, g-major weights; ~45 DMAs/exec
instead of 270).  Measured steady-state body (hardware For_i loop delta):
~476 us/exec -> 68.7 GFLOP/core at ~144 TF/s = 92% of the 157 TF/s fp8
peak, i.e. at the DoubleRow compute roofline.  (The shared axon devices'
absolute speed drifts ~30% across time windows; relative comparisons within
a measurement batch were used for all structural decisions.)
"""

import os
import numpy as np
import ml_dtypes

B, D, E, U = 4096, 1024, 8, 4096
BT = 512              # token tile (matmul free dim)
NB = B // BT          # 8 token tiles
DC = D // 128         # 8 chunks of the model dim
UC = U // 128         # 32 chunks of the hidden dim
N_CORES = 8
P = 128
WSCALE = 64.0         # host pre-scale on W1/W2/Wr so fp8e4 values are ~N(0,1..2)

_BF16 = ml_dtypes.bfloat16
_F8 = ml_dtypes.float8_e4m3   # TRN fp8e4: max normal +-240, then +-inf

_NC_CACHE = {}
LAST_RESULTS = None


def _build_nc(bench_loop=0, has_b1=False, pair_act=False, swap=True,
              ps1_bufs=2, psr_bufs=1, out_split=2, dr_router=True):
    import concourse.mybir as mybir
    import concourse.tile as tile
    from concourse import bacc

    f32 = mybir.dt.float32
    bf16 = mybir.dt.bfloat16
    f8 = mybir.dt.float8e4
    AF = mybir.ActivationFunctionType
    ALU = mybir.AluOpType
    DR = mybir.MatmulPerfMode.DoubleRow

    nc = bacc.Bacc("TRN2", target_bir_lowering=False, debug=False,
                   num_devices=N_CORES)

    # host-side layouts are chosen so every DMA is per-partition CONTIGUOUS
    # (2-4KB descriptors): bt-major for x/out, g-major for weights
    xt8 = nc.dram_tensor("xt8", [NB, P, DC * BT], f8, kind="ExternalInput").ap()
    w1 = nc.dram_tensor("w1", [4, P, DC * 1024], f8, kind="ExternalInput").ap()
    w2 = nc.dram_tensor("w2", [4, P, 8 * D], f8, kind="ExternalInput").ap()
    EP = 16                       # router weight padded to 16 for DoubleRow
    wr = nc.dram_tensor("wr", [P, DC, EP], f8, kind="ExternalInput").ap()
    bp = nc.dram_tensor("bp", [P, UC + DC], f32, kind="ExternalInput").ap()
    br = nc.dram_tensor("br", [E, 1], f32, kind="ExternalInput").ap()
    selc = nc.dram_tensor("selc", [E, P + 1], bf16, kind="ExternalInput").ap()
    o18 = nc.dram_tensor("o18", [1, E], bf16, kind="ExternalInput").ap()
    o = nc.dram_tensor("o", [NB, P, DC * BT], bf16, kind="ExternalOutput").ap()

    with tile.TileContext(nc) as tc:
        with (
            tc.tile_pool(name="wp", bufs=1) as wp,
            tc.tile_pool(name="x8p", bufs=NB) as x8p,
            tc.tile_pool(name="hbp", bufs=2) as hbp,
            tc.tile_pool(name="r8p", bufs=4) as r8p,
            tc.tile_pool(name="r1p", bufs=2) as r1p,
            tc.tile_pool(name="scp", bufs=NB) as scp,
            tc.tile_pool(name="ctp", bufs=2) as ctp,
            tc.tile_pool(name="ps1p", bufs=ps1_bufs, space="PSUM") as ps1p,
            tc.tile_pool(name="ps2p", bufs=2, space="PSUM") as ps2p,
            tc.tile_pool(name="psrp", bufs=psr_bufs, space="PSUM") as psrp,
            tc.tile_pool(name="pssp", bufs=1, space="PSUM") as pssp,
        ):
            w1t = wp.tile([P, 4, DC, 1024], f8)   # [p, g, dc, j]
            w2t = wp.tile([P, 4, 8, D], f8)       # [p, cgrp, r, j], uc=8*cgrp+r
            wrt = wp.tile([P, DC, EP], f8)
            bpt = wp.tile([P, UC + DC], f32)
            brt = wp.tile([E, 1], f32)
            selt = wp.tile([E, P + 1], bf16)
            o18t = wp.tile([1, E], bf16)
            selbt = selt[:, 0:P]
            o8t = selt[:, P : P + 1]

            def emit_x8(bt):
                x8 = x8p.tile([P, DC, BT], f8, tag="x8")
                half = DC * BT // 2
                for h in (0, 1):
                    nc.sync.dma_start(
                        out=x8[:, 4 * h : 4 * h + 4, :],
                        in_=xt8[bt, :, h * half : (h + 1) * half])
                return x8

            def emit_router(x8):
                # weights = softmax(softmax(x@Wr + br)), gate >0.1, row e
                # broadcast to 128 partitions (scaled by 1/WSCALE for mm2).
                # Softmax chain in bf16 so the 8-row sum / broadcast matmuls
                # run at 1 cycle/row instead of fp32's 4.
                lg = psrp.tile([EP if dr_router else E, BT], f32, tag="rps")
                if dr_router:
                    for dk in range(DC // 2):
                        nc.tensor.matmul(
                            lg[:], wrt[:, 2 * dk : 2 * dk + 2, :],
                            x8[:, 2 * dk : 2 * dk + 2, :],
                            start=(dk == 0), stop=(dk == DC // 2 - 1),
                            perf_mode=DR,
                        )
                else:
                    for dc in range(DC):
                        nc.tensor.matmul(
                            lg[:], wrt[:, dc, 0:E], x8[:, dc, :],
                            start=(dc == 0), stop=(dc == DC - 1),
                        )
                t1 = r8p.tile([E, BT], bf16, tag="r8")
                nc.scalar.activation(t1[:], lg[0:E, :], AF.Exp,
                                     bias=brt[:, 0:1], scale=1.0 / WSCALE)
                s1 = psrp.tile([1, BT], f32, tag="rps")
                nc.tensor.matmul(s1[:], o8t, t1[:], start=True, stop=True)
                r1 = r1p.tile([1, BT], bf16, tag="r1")
                with nc.allow_low_precision(reason="router softmax sums; "
                                            ">8x margin to the 0.1 gate"):
                    nc.vector.reciprocal(r1[:], s1[:])
                rb1 = psrp.tile([E, BT], f32, tag="rps")
                nc.tensor.matmul(rb1[:], o18t[:], r1[:], start=True, stop=True)
                pp = r8p.tile([E, BT], bf16, tag="r8")
                nc.vector.tensor_tensor(pp[:], t1[:], rb1[:], ALU.mult)
                t2 = r8p.tile([E, BT], bf16, tag="r8")
                nc.scalar.activation(t2[:], pp[:], AF.Exp)
                s2 = psrp.tile([1, BT], f32, tag="rps")
                nc.tensor.matmul(s2[:], o8t, t2[:], start=True, stop=True)
                r2 = r1p.tile([1, BT], bf16, tag="r1")
                with nc.allow_low_precision(reason="router softmax sums; "
                                            ">8x margin to the 0.1 gate"):
                    nc.vector.reciprocal(r2[:], s2[:])
                rb2 = psrp.tile([E, BT], f32, tag="rps")
                nc.tensor.matmul(rb2[:], o18t[:], r2[:], start=True, stop=True)
                wg = r8p.tile([E, BT], bf16, tag="r8")
                nc.vector.tensor_tensor(wg[:], t2[:], rb2[:], ALU.mult)
                sc = r8p.tile([E, BT], bf16, tag="r8")
                nc.vector.scalar_tensor_tensor(
                    sc[:], wg[:], 0.1, wg[:], ALU.is_gt, ALU.mult
                )
                s128ps = pssp.tile([P, BT], f32, tag="pss")
                nc.tensor.matmul(s128ps[:], selbt, sc[:], start=True, stop=True)
                s128 = scp.tile([P, BT], f32, tag="s128")
                nc.vector.tensor_copy(s128[:], s128ps[:])
                return s128

            def emit_mm1(x8):
                # h^T = swish((W1*64)^T x^T / 64 + b1), fp8 DoubleRow,
                # pair-fused PSUM + activation
                h8 = hbp.tile([P, UC, BT], f8, tag="hb")
                if pair_act:
                    for up in range(UC // 2):
                        ps1 = ps1p.tile([P, 2, BT], f32, tag="ps1")
                        for h in (0, 1):
                            uc = 2 * up + h
                            g, r = uc >> 3, uc & 7
                            for dk in range(DC // 2):
                                nc.tensor.matmul(
                                    ps1[:, h, :],
                                    w1t[:, g, 2 * dk : 2 * dk + 2,
                                        r * P : (r + 1) * P],
                                    x8[:, 2 * dk : 2 * dk + 2, :],
                                    start=(dk == 0), stop=(dk == DC // 2 - 1),
                                    perf_mode=DR,
                                )
                        if has_b1:
                            for h in (0, 1):
                                uc = 2 * up + h
                                nc.scalar.activation(
                                    h8[:, uc, :], ps1[:, h, :], AF.Silu,
                                    bias=bpt[:, uc : uc + 1],
                                    scale=1.0 / WSCALE)
                        else:
                            nc.scalar.activation(
                                h8[:, 2 * up : 2 * up + 2, :], ps1[:, :, :],
                                AF.Silu, scale=1.0 / WSCALE)
                else:
                    for uc in range(UC):
                        ps1 = ps1p.tile([P, BT], f32, tag="ps1")
                        g, r = uc >> 3, uc & 7
                        for dk in range(DC // 2):
                            nc.tensor.matmul(
                                ps1[:],
                                w1t[:, g, 2 * dk : 2 * dk + 2,
                                    r * P : (r + 1) * P],
                                x8[:, 2 * dk : 2 * dk + 2, :],
                                start=(dk == 0), stop=(dk == DC // 2 - 1),
                                perf_mode=DR,
                            )
                        nc.scalar.activation(
                            h8[:, uc, :], ps1[:], AF.Silu,
                            bias=bpt[:, uc : uc + 1], scale=1.0 / WSCALE)
                return h8

            def emit_mm2(bt, h8, s128):
                # contrib^T = ((W2*64)^T h^T + 64*b2) * (w*mask/64), bf16 out
                ct = ctp.tile([P, DC, BT], bf16, tag="ct")
                for dc in range(DC):
                    ps2 = ps2p.tile([P, BT], f32, tag="ps2")
                    for uk in range(UC // 2):
                        cg, r = (2 * uk) >> 3, (2 * uk) & 7
                        nc.tensor.matmul(
                            ps2[:],
                            w2t[:, cg, r : r + 2, dc * P : (dc + 1) * P],
                            h8[:, 2 * uk : 2 * uk + 2, :],
                            start=(uk == 0), stop=(uk == UC // 2 - 1),
                            perf_mode=DR,
                        )
                    nc.vector.scalar_tensor_tensor(
                        ct[:, dc, :], ps2[:], bpt[:, UC + dc : UC + dc + 1],
                        s128[:], ALU.add, ALU.mult,
                    )
                chunk = DC // out_split
                seg = chunk * BT
                for h in range(out_split):
                    nc.sync.dma_start(
                        out=o[bt, :, h * seg : (h + 1) * seg],
                        in_=ct[:, chunk * h : chunk * (h + 1), :])

            def dma_w(wt, wsrc, g):
                half = wsrc.shape[-1] // 2
                for h in (0, 1):
                    nc.sync.dma_start(
                        out=wt[:, g, 4 * h : 4 * h + 4, :],
                        in_=wsrc[g, :, h * half : (h + 1) * half])

            def emit_consts():
                nc.sync.dma_start(out=selt[:], in_=selc[:])
                nc.sync.dma_start(out=brt[:], in_=br[:])
                nc.sync.dma_start(out=o18t[:], in_=o18[:])
                nc.sync.dma_start(out=wrt[:, :, :], in_=wr[:, :, :])
                nc.sync.dma_start(out=bpt[:], in_=bp[:])

            def emit_main(skip_weights=False):
                # DMA-queue order is emission order: interleave x loads with
                # weight chunks so each lands just before first use.  ALL 8
                # routers run up front: their PE work hides in the DMA ramp
                # and ScalarE pays ~2 Exp<->Silu table switches total instead
                # of 4 per token tile.
                if not skip_weights:
                    emit_consts()
                x8s = []
                for bt in range(NB):
                    x8s.append(emit_x8(bt))
                    if not skip_weights and bt % 2 == 1 and bt // 2 < 4:
                        dma_w(w1t, w1, bt // 2)
                s128s = [emit_router(x8s[bt]) for bt in range(NB)]
                if not skip_weights:
                    for cgroup in range(4):
                        dma_w(w2t, w2, cgroup)

                h8_cur = emit_mm1(x8s[0])
                for bt in range(NB):
                    if swap:
                        if bt + 1 < NB:
                            h8_next = emit_mm1(x8s[bt + 1])
                        emit_mm2(bt, h8_cur, s128s[bt])
                        if bt + 1 < NB:
                            h8_cur = h8_next
                    else:
                        emit_mm2(bt, h8_cur, s128s[bt])
                        if bt + 1 < NB:
                            h8_cur = emit_mm1(x8s[bt + 1])

            if bench_loop:
                emit_consts()
                for g in range(4):
                    dma_w(w1t, w1, g)
                for cgroup in range(4):
                    dma_w(w2t, w2, cgroup)
                with tc.For_i(0, bench_loop, 1):
                    emit_main(skip_weights=True)
            else:
                emit_main()

    nc.compile()
    return nc


def _get_nc():
    if "nc" not in _NC_CACHE:
        _NC_CACHE["nc"] = _build_nc()
    return _NC_CACHE["nc"]


def _f8(a):
    return np.clip(a, -240.0, 240.0).astype(_F8)


def _chunked(a, nchunk):
    # [rows, free] -> [128, nchunk, free] with row = chunk*128 + partition
    rows, free = a.shape
    return np.ascontiguousarray(
        a.reshape(nchunk, P, free).transpose(1, 0, 2))


def _prep_in_maps(inputs):
    x = np.asarray(inputs["x"], np.float32)
    Wr = np.asarray(inputs["Wr"], np.float32)
    br = np.asarray(inputs["br"], np.float32)
    W1 = np.asarray(inputs["W1"], np.float32)
    b1 = np.asarray(inputs["b1"], np.float32)
    W2 = np.asarray(inputs["W2"], np.float32)
    b2 = np.asarray(inputs["b2"], np.float32)

    # x: [P, DC, B] -> bt-major [NB, P, DC*BT] so each bt's DMA is contiguous
    xt8 = _f8(_chunked(np.ascontiguousarray(x.T), DC)
              .reshape(P, DC, NB, BT).transpose(2, 0, 1, 3)
              .reshape(NB, P, DC * BT))
    wr_pad = np.zeros((D, 16), np.float32)
    wr_pad[:, 0:E] = Wr * WSCALE
    wr8 = _f8(_chunked(wr_pad, DC))
    br_c = np.ascontiguousarray(br.reshape(E, 1))
    o18_c = np.ones((1, E), _BF16)

    in_maps = []
    for c in range(N_CORES):
        selc = np.zeros((E, P + 1), np.float32)
        selc[c, 0:P] = 1.0 / WSCALE
        selc[:, P] = 1.0           # the all-ones column for row sums
        bpk = np.concatenate(
            [b1[c].reshape(UC, P).T, b2[c].reshape(DC, P).T * WSCALE],
            axis=1)
        # weights g-major: [4, P, chunk*1024] contiguous per partition
        w1c = (_chunked(W1[c], DC).reshape(P, DC, 4, 1024)
               .transpose(2, 0, 1, 3).reshape(4, P, DC * 1024))
        w2c = (_chunked(W2[c], UC).reshape(P, 4, 8, D)
               .transpose(1, 0, 2, 3).reshape(4, P, 8 * D))
        in_maps.append({
            "xt8": xt8,
            "w1": _f8(w1c * WSCALE),
            "w2": _f8(w2c * WSCALE),
            "wr": wr8,
            "bp": np.ascontiguousarray(bpk),
            "br": br_c,
            "selc": selc.astype(_BF16),
            "o18": o18_c,
        })
    return in_maps


def kernel(**inputs):
    from concourse.bass_utils import run_bass_kernel_spmd

    global LAST_RESULTS

    in_maps = _prep_in_maps(inputs)
    nc = _get_nc()
    want_trace = bool(int(os.environ.get("KERNEL_TRACE", "0")))
    if not want_trace:
        # the NTFF-trace path needs antenv.axon_hooks, which this container
        # lacks; make sure a stray BASS_TRACE env can't route us into it
        os.environ["BASS_NEVER_TRACE"] = "1"
    res = run_bass_kernel_spmd(
        nc, in_maps, core_ids=list(range(N_CORES)), trace=want_trace,
    )
    LAST_RESULTS = res

    # host: 8-way partial-sum reduction + residual + layout transform
    acc = res.results[0]["o"].astype(np.float32)
    for c in range(1, N_CORES):
        acc += res.results[c]["o"].astype(np.float32)
    # acc[bt, p, dc*BT+t] -> out[bt*BT+t, dc*128+p]
    out = (acc.reshape(NB, P, DC, BT).transpose(0, 3, 2, 1).reshape(B, D)
           + np.asarray(inputs["x"], np.float32))
    return np.ascontiguousarray(out)


# revision 54
# speedup vs baseline: 1.1010x; 1.1010x over previous
"""MoE (dense all-expert FFN with double-softmax routing) on 8 trn2 NeuronCores.

Expert-parallel: core c holds expert c's W1/W2/b1/b2 resident in SBUF (fp8e4,
pre-scaled x64 on host for precision) and computes its expert's routing-
weighted contribution
    contrib_c = weight_c * mask_c * (swish(x @ W1[c] + b1[c]) @ W2[c] + b2[c])
for all 4096 tokens, written transposed as [128, 8, 4096] bf16.  The host
gathers the 8 partial outputs and forms  sum_c(contrib_c)^T + x  (a pure
8-way reduction + residual + layout transform; all matmuls / softmaxes /
activations / masking run on device).

Both big matmuls run in fp8 DoubleRow mode (2 fp8 weights per PE cell,
contracting 256 rows per instruction, ~2x bf16 throughput).  The x64 weight
scale is undone by the activation `scale` for mm1 and folded into the
routing-weight broadcast for mm2.  The router also runs on the fp8 x / Wr
(validated: identical >0.1 mask as the f32 reference, with ~8x margin) with a
bf16 softmax chain; the weighted combine multiplier stays f32.

Structure per token tile bt (512 tokens): the kernel is software-pipelined as
  router(bt+1), mm1(bt+1) -> h8(bt+1)   then   mm2(bt) -> combine -> out DMA
so the ScalarE activation backlog of mm1(bt+1) drains during mm2(bt)'s PE
work instead of stalling it.  The router's logit matmul also runs DoubleRow
(Wr zero-padded 8 -> 16 experts so the weight AP middle step is 16 bytes).

All tensors are staged host-side so every DMA is per-partition contiguous
with 2-4KB descriptors (bt-major x/out, g-major weights; ~45 DMAs/exec
instead of 270).  Measured steady-state body (hardware For_i loop delta):
~476 us/exec -> 68.7 GFLOP/core at ~144 TF/s = 92% of the 157 TF/s fp8
peak, i.e. at the DoubleRow compute roofline.  (The shared axon devices'
absolute speed drifts ~30% across time windows; relative comparisons within
a measurement batch were used for all structural decisions.)
"""

import os
import numpy as np
import ml_dtypes

B, D, E, U = 4096, 1024, 8, 4096
BT = 512              # token tile (matmul free dim)
NB = B // BT          # 8 token tiles
DC = D // 128         # 8 chunks of the model dim
UC = U // 128         # 32 chunks of the hidden dim
N_CORES = 8
P = 128
WSCALE = 64.0         # host pre-scale on W1/W2/Wr so fp8e4 values are ~N(0,1..2)

_BF16 = ml_dtypes.bfloat16
_F8 = ml_dtypes.float8_e4m3   # TRN fp8e4: max normal +-240, then +-inf

_NC_CACHE = {}
LAST_RESULTS = None


def _build_nc(bench_loop=0, has_b1=False, pair_act=False, swap=True,
              ps1_bufs=2, psr_bufs=1, out_split=2, dr_router=True,
              routers_front=False, staged=True):
    import concourse.mybir as mybir
    import concourse.tile as tile
    from concourse import bacc

    f32 = mybir.dt.float32
    bf16 = mybir.dt.bfloat16
    f8 = mybir.dt.float8e4
    AF = mybir.ActivationFunctionType
    ALU = mybir.AluOpType
    DR = mybir.MatmulPerfMode.DoubleRow

    nc = bacc.Bacc("TRN2", target_bir_lowering=False, debug=False,
                   num_devices=N_CORES)

    # host-side layouts are chosen so every DMA is per-partition CONTIGUOUS
    # (2-4KB descriptors): bt-major for x/out, g-major for weights
    xt8 = nc.dram_tensor("xt8", [NB, P, DC * BT], f8, kind="ExternalInput").ap()
    w1 = nc.dram_tensor("w1", [4, P, DC * 1024], f8, kind="ExternalInput").ap()
    w2 = nc.dram_tensor("w2", [4, P, 8 * D], f8, kind="ExternalInput").ap()
    EP = 16                       # router weight padded to 16 for DoubleRow
    wr = nc.dram_tensor("wr", [P, DC, EP], f8, kind="ExternalInput").ap()
    bp = nc.dram_tensor("bp", [P, UC + DC], f32, kind="ExternalInput").ap()
    br = nc.dram_tensor("br", [E, 1], f32, kind="ExternalInput").ap()
    selc = nc.dram_tensor("selc", [E, P + 1], bf16, kind="ExternalInput").ap()
    o18 = nc.dram_tensor("o18", [1, E], bf16, kind="ExternalInput").ap()
    o = nc.dram_tensor("o", [NB, P, DC * BT], bf16, kind="ExternalOutput").ap()

    with tile.TileContext(nc) as tc:
        with (
            tc.tile_pool(name="wp", bufs=1) as wp,
            tc.tile_pool(name="x8p", bufs=NB) as x8p,
            tc.tile_pool(name="hbp", bufs=2) as hbp,
            tc.tile_pool(name="r8p", bufs=4) as r8p,
            tc.tile_pool(name="r1p", bufs=2) as r1p,
            tc.tile_pool(name="scp", bufs=NB) as scp,
            tc.tile_pool(name="ctp", bufs=2) as ctp,
            tc.tile_pool(name="ps1p", bufs=ps1_bufs, space="PSUM") as ps1p,
            tc.tile_pool(name="ps2p", bufs=2, space="PSUM") as ps2p,
            tc.tile_pool(name="psrp", bufs=psr_bufs, space="PSUM") as psrp,
            tc.tile_pool(name="pssp", bufs=1, space="PSUM") as pssp,
        ):
            w1t = wp.tile([P, 4, DC, 1024], f8)   # [p, g, dc, j]
            w2t = wp.tile([P, 4, 8, D], f8)       # [p, cgrp, r, j], uc=8*cgrp+r
            wrt = wp.tile([P, DC, EP], f8)
            bpt = wp.tile([P, UC + DC], f32)
            brt = wp.tile([E, 1], f32)
            selt = wp.tile([E, P + 1], bf16)
            o18t = wp.tile([1, E], bf16)
            selbt = selt[:, 0:P]
            o8t = selt[:, P : P + 1]

            def emit_x8(bt):
                x8 = x8p.tile([P, DC, BT], f8, tag="x8")
                half = DC * BT // 2
                for h in (0, 1):
                    nc.sync.dma_start(
                        out=x8[:, 4 * h : 4 * h + 4, :],
                        in_=xt8[bt, :, h * half : (h + 1) * half])
                return x8

            def emit_router_stages(x8):
                # weights = softmax(softmax(x@Wr + br)), gate >0.1, row e
                # broadcast to 128 partitions (scaled by 1/WSCALE for mm2).
                # Softmax chain in bf16 so the 8-row sum / broadcast matmuls
                # run at 1 cycle/row instead of fp32's 4.
                # Returns (s128, [stage closures]): stage 0 is the logit
                # matmul; later stages each start with a PE matmul that waits
                # on a ScalarE/DVE result.  The caller interleaves stages
                # between mm1 matmul groups so those cross-engine waits are
                # resolved before the PE queue reaches them (otherwise each
                # router costs ~3us of in-order PE-queue bubbles).
                s128 = scp.tile([P, BT], f32, tag="s128")
                st = {}

                def stage0():
                    lg = psrp.tile([EP if dr_router else E, BT], f32,
                                   tag="rps")
                    st["lg"] = lg
                    if dr_router:
                        for dk in range(DC // 2):
                            nc.tensor.matmul(
                                lg[:], wrt[:, 2 * dk : 2 * dk + 2, :],
                                x8[:, 2 * dk : 2 * dk + 2, :],
                                start=(dk == 0), stop=(dk == DC // 2 - 1),
                                perf_mode=DR,
                            )
                    else:
                        for dc in range(DC):
                            nc.tensor.matmul(
                                lg[:], wrt[:, dc, 0:E], x8[:, dc, :],
                                start=(dc == 0), stop=(dc == DC - 1),
                            )
                    t1 = r8p.tile([E, BT], bf16, tag="r8")
                    nc.scalar.activation(t1[:], lg[0:E, :], AF.Exp,
                                         bias=brt[:, 0:1], scale=1.0 / WSCALE)
                    st["t1"] = t1

                def stage1():
                    s1 = psrp.tile([1, BT], f32, tag="rps")
                    nc.tensor.matmul(s1[:], o8t, st["t1"][:],
                                     start=True, stop=True)
                    r1 = r1p.tile([1, BT], bf16, tag="r1")
                    with nc.allow_low_precision(
                            reason="router softmax sums; >8x margin"):
                        nc.vector.reciprocal(r1[:], s1[:])
                    st["r1"] = r1

                def stage2():
                    rb1 = psrp.tile([E, BT], f32, tag="rps")
                    nc.tensor.matmul(rb1[:], o18t[:], st["r1"][:],
                                     start=True, stop=True)
                    pp = r8p.tile([E, BT], bf16, tag="r8")
                    nc.vector.tensor_tensor(pp[:], st["t1"][:], rb1[:],
                                            ALU.mult)
                    t2 = r8p.tile([E, BT], bf16, tag="r8")
                    nc.scalar.activation(t2[:], pp[:], AF.Exp)
                    st["t2"] = t2

                def stage3():
                    s2 = psrp.tile([1, BT], f32, tag="rps")
                    nc.tensor.matmul(s2[:], o8t, st["t2"][:],
                                     start=True, stop=True)
                    r2 = r1p.tile([1, BT], bf16, tag="r1")
                    with nc.allow_low_precision(
                            reason="router softmax sums; >8x margin"):
                        nc.vector.reciprocal(r2[:], s2[:])
                    st["r2"] = r2

                def stage4():
                    rb2 = psrp.tile([E, BT], f32, tag="rps")
                    nc.tensor.matmul(rb2[:], o18t[:], st["r2"][:],
                                     start=True, stop=True)
                    wg = r8p.tile([E, BT], bf16, tag="r8")
                    nc.vector.tensor_tensor(wg[:], st["t2"][:], rb2[:],
                                            ALU.mult)
                    sc = r8p.tile([E, BT], bf16, tag="r8")
                    nc.vector.scalar_tensor_tensor(
                        sc[:], wg[:], 0.1, wg[:], ALU.is_gt, ALU.mult)
                    st["sc"] = sc

                def stage5():
                    s128ps = pssp.tile([P, BT], f32, tag="pss")
                    nc.tensor.matmul(s128ps[:], selbt, st["sc"][:],
                                     start=True, stop=True)
                    nc.vector.tensor_copy(s128[:], s128ps[:])

                return s128, [stage0, stage1, stage2, stage3, stage4, stage5]

            def emit_router(x8):
                s128, stages = emit_router_stages(x8)
                for s in stages:
                    s()
                return s128

            def emit_mm1(x8, interleave=None):
                # h^T = swish((W1*64)^T x^T / 64 + b1), fp8 DoubleRow,
                # pair-fused PSUM + activation
                if interleave is None:
                    interleave = []
                h8 = hbp.tile([P, UC, BT], f8, tag="hb")
                if pair_act:
                    for up in range(UC // 2):
                        ps1 = ps1p.tile([P, 2, BT], f32, tag="ps1")
                        for h in (0, 1):
                            uc = 2 * up + h
                            g, r = uc >> 3, uc & 7
                            for dk in range(DC // 2):
                                nc.tensor.matmul(
                                    ps1[:, h, :],
                                    w1t[:, g, 2 * dk : 2 * dk + 2,
                                        r * P : (r + 1) * P],
                                    x8[:, 2 * dk : 2 * dk + 2, :],
                                    start=(dk == 0), stop=(dk == DC // 2 - 1),
                                    perf_mode=DR,
                                )
                        if has_b1:
                            for h in (0, 1):
                                uc = 2 * up + h
                                nc.scalar.activation(
                                    h8[:, uc, :], ps1[:, h, :], AF.Silu,
                                    bias=bpt[:, uc : uc + 1],
                                    scale=1.0 / WSCALE)
                        else:
                            nc.scalar.activation(
                                h8[:, 2 * up : 2 * up + 2, :], ps1[:, :, :],
                                AF.Silu, scale=1.0 / WSCALE)
                else:
                    for uc in range(UC):
                        ps1 = ps1p.tile([P, BT], f32, tag="ps1")
                        g, r = uc >> 3, uc & 7
                        for dk in range(DC // 2):
                            nc.tensor.matmul(
                                ps1[:],
                                w1t[:, g, 2 * dk : 2 * dk + 2,
                                    r * P : (r + 1) * P],
                                x8[:, 2 * dk : 2 * dk + 2, :],
                                start=(dk == 0), stop=(dk == DC // 2 - 1),
                                perf_mode=DR,
                            )
                        nc.scalar.activation(
                            h8[:, uc, :], ps1[:], AF.Silu,
                            bias=bpt[:, uc : uc + 1], scale=1.0 / WSCALE)
                        # inject a pending router stage every 4 uc groups
                        # (~3.6us of PE work between stages hides each
                        # cross-engine dependency)
                        if interleave and uc % 4 == 3:
                            interleave.pop(0)()
                while interleave:
                    interleave.pop(0)()
                return h8

            def emit_mm2(bt, h8, s128):
                # contrib^T = ((W2*64)^T h^T + 64*b2) * (w*mask/64), bf16 out
                ct = ctp.tile([P, DC, BT], bf16, tag="ct")
                for dc in range(DC):
                    ps2 = ps2p.tile([P, BT], f32, tag="ps2")
                    for uk in range(UC // 2):
                        cg, r = (2 * uk) >> 3, (2 * uk) & 7
                        nc.tensor.matmul(
                            ps2[:],
                            w2t[:, cg, r : r + 2, dc * P : (dc + 1) * P],
                            h8[:, 2 * uk : 2 * uk + 2, :],
                            start=(uk == 0), stop=(uk == UC // 2 - 1),
                            perf_mode=DR,
                        )
                    nc.vector.scalar_tensor_tensor(
                        ct[:, dc, :], ps2[:], bpt[:, UC + dc : UC + dc + 1],
                        s128[:], ALU.add, ALU.mult,
                    )
                chunk = DC // out_split
                seg = chunk * BT
                for h in range(out_split):
                    nc.sync.dma_start(
                        out=o[bt, :, h * seg : (h + 1) * seg],
                        in_=ct[:, chunk * h : chunk * (h + 1), :])

            def dma_w(wt, wsrc, g):
                half = wsrc.shape[-1] // 2
                for h in (0, 1):
                    nc.sync.dma_start(
                        out=wt[:, g, 4 * h : 4 * h + 4, :],
                        in_=wsrc[g, :, h * half : (h + 1) * half])

            def emit_consts():
                nc.sync.dma_start(out=selt[:], in_=selc[:])
                nc.sync.dma_start(out=brt[:], in_=br[:])
                nc.sync.dma_start(out=o18t[:], in_=o18[:])
                nc.sync.dma_start(out=wrt[:, :, :], in_=wr[:, :, :])
                nc.sync.dma_start(out=bpt[:], in_=bp[:])

            def emit_main(skip_weights=False):
                # DMA-queue order is emission order: interleave x loads with
                # weight chunks so each lands just before first use.
                # routers_front=True bunches all 8 routers in a prologue
                # (fewer ACT table switches) but measured ~35us slower on HW
                # than distributing them, so it is off by default.
                if not skip_weights:
                    emit_consts()
                if routers_front:
                    x8s = []
                    for bt in range(NB):
                        x8s.append(emit_x8(bt))
                        if not skip_weights and bt % 2 == 1 and bt // 2 < 4:
                            dma_w(w1t, w1, bt // 2)
                    s128s = [emit_router(x8s[bt]) for bt in range(NB)]
                    if not skip_weights:
                        for cgroup in range(4):
                            dma_w(w2t, w2, cgroup)

                    h8_cur = emit_mm1(x8s[0])
                    for bt in range(NB):
                        if swap:
                            if bt + 1 < NB:
                                h8_next = emit_mm1(x8s[bt + 1])
                            emit_mm2(bt, h8_cur, s128s[bt])
                            if bt + 1 < NB:
                                h8_cur = h8_next
                        else:
                            emit_mm2(bt, h8_cur, s128s[bt])
                            if bt + 1 < NB:
                                h8_cur = emit_mm1(x8s[bt + 1])
                    return
                # distributed-router variant: router(bt+1)'s logit matmul is
                # emitted first, then its dependent PE steps are interleaved
                # between mm1(bt+1)'s matmul groups so the in-order PE queue
                # never blocks on a ScalarE/DVE router result.
                def router_and_mm1(x8):
                    if staged:
                        s128, stg = emit_router_stages(x8)
                        stg[0]()
                        h8 = emit_mm1(x8, interleave=stg[1:])
                    else:
                        s128 = emit_router(x8)
                        h8 = emit_mm1(x8)
                    return s128, h8

                x8_cur = emit_x8(0)
                if not staged:
                    s128_cur = emit_router(x8_cur)
                else:
                    s128_cur, stg0 = emit_router_stages(x8_cur)
                    stg0[0]()
                if not skip_weights:
                    dma_w(w1t, w1, 0)
                    dma_w(w1t, w1, 1)
                x8_next = emit_x8(1)
                if not skip_weights:
                    dma_w(w1t, w1, 2)
                    dma_w(w1t, w1, 3)
                x8_far = emit_x8(2)
                h8_cur = emit_mm1(x8_cur,
                                  interleave=stg0[1:] if staged else None)
                if not skip_weights:
                    for cgroup in range(4):
                        dma_w(w2t, w2, cgroup)
                for bt in range(NB):
                    if bt + 1 < NB:
                        if bt + 3 < NB:
                            x8_new = emit_x8(bt + 3)
                        s128_next, h8_next = router_and_mm1(x8_next)
                        x8_next = x8_far
                        x8_far = x8_new if bt + 3 < NB else None
                    emit_mm2(bt, h8_cur, s128_cur)
                    if bt + 1 < NB:
                        h8_cur, s128_cur = h8_next, s128_next

            if bench_loop:
                emit_consts()
                for g in range(4):
                    dma_w(w1t, w1, g)
                for cgroup in range(4):
                    dma_w(w2t, w2, cgroup)
                with tc.For_i(0, bench_loop, 1):
                    emit_main(skip_weights=True)
            else:
                emit_main()

    nc.compile()
    return nc


def _get_nc():
    if "nc" not in _NC_CACHE:
        _NC_CACHE["nc"] = _build_nc()
    return _NC_CACHE["nc"]


def _f8(a):
    return np.clip(a, -240.0, 240.0).astype(_F8)


def _chunked(a, nchunk):
    # [rows, free] -> [128, nchunk, free] with row = chunk*128 + partition
    rows, free = a.shape
    return np.ascontiguousarray(
        a.reshape(nchunk, P, free).transpose(1, 0, 2))


def _prep_in_maps(inputs):
    x = np.asarray(inputs["x"], np.float32)
    Wr = np.asarray(inputs["Wr"], np.float32)
    br = np.asarray(inputs["br"], np.float32)
    W1 = np.asarray(inputs["W1"], np.float32)
    b1 = np.asarray(inputs["b1"], np.float32)
    W2 = np.asarray(inputs["W2"], np.float32)
    b2 = np.asarray(inputs["b2"], np.float32)

    # x: [P, DC, B] -> bt-major [NB, P, DC*BT] so each bt's DMA is contiguous
    xt8 = _f8(_chunked(np.ascontiguousarray(x.T), DC)
              .reshape(P, DC, NB, BT).transpose(2, 0, 1, 3)
              .reshape(NB, P, DC * BT))
    wr_pad = np.zeros((D, 16), np.float32)
    wr_pad[:, 0:E] = Wr * WSCALE
    wr8 = _f8(_chunked(wr_pad, DC))
    br_c = np.ascontiguousarray(br.reshape(E, 1))
    o18_c = np.ones((1, E), _BF16)

    in_maps = []
    for c in range(N_CORES):
        selc = np.zeros((E, P + 1), np.float32)
        selc[c, 0:P] = 1.0 / WSCALE
        selc[:, P] = 1.0           # the all-ones column for row sums
        bpk = np.concatenate(
            [b1[c].reshape(UC, P).T, b2[c].reshape(DC, P).T * WSCALE],
            axis=1)
        # weights g-major: [4, P, chunk*1024] contiguous per partition
        w1c = (_chunked(W1[c], DC).reshape(P, DC, 4, 1024)
               .transpose(2, 0, 1, 3).reshape(4, P, DC * 1024))
        w2c = (_chunked(W2[c], UC).reshape(P, 4, 8, D)
               .transpose(1, 0, 2, 3).reshape(4, P, 8 * D))
        in_maps.append({
            "xt8": xt8,
            "w1": _f8(w1c * WSCALE),
            "w2": _f8(w2c * WSCALE),
            "wr": wr8,
            "bp": np.ascontiguousarray(bpk),
            "br": br_c,
            "selc": selc.astype(_BF16),
            "o18": o18_c,
        })
    return in_maps


def kernel(**inputs):
    from concourse.bass_utils import run_bass_kernel_spmd

    global LAST_RESULTS

    in_maps = _prep_in_maps(inputs)
    nc = _get_nc()
    want_trace = bool(int(os.environ.get("KERNEL_TRACE", "0")))
    if not want_trace:
        # the NTFF-trace path needs antenv.axon_hooks, which this container
        # lacks; make sure a stray BASS_TRACE env can't route us into it
        os.environ["BASS_NEVER_TRACE"] = "1"
    res = run_bass_kernel_spmd(
        nc, in_maps, core_ids=list(range(N_CORES)), trace=want_trace,
    )
    LAST_RESULTS = res

    # host: 8-way partial-sum reduction + residual + layout transform
    acc = res.results[0]["o"].astype(np.float32)
    for c in range(1, N_CORES):
        acc += res.results[c]["o"].astype(np.float32)
    # acc[bt, p, dc*BT+t] -> out[bt*BT+t, dc*128+p]
    out = (acc.reshape(NB, P, DC, BT).transpose(0, 3, 2, 1).reshape(B, D)
           + np.asarray(inputs["x"], np.float32))
    return np.ascontiguousarray(out)
